# revision 12
# baseline (speedup 1.0000x reference)
"""Trainium2 Bass kernel for nn_PositionalEncoding.

The reference module's realized math discards the input entirely:
    out = broadcast_to(2 * pe_table, x.shape)   # pe_table is [64, 16]
so the kernel is pure output-bandwidth bound. Each of the 8 cores writes
its (8192, 64, 16) f32 shard (32 MiB) of the broadcast table. The input
`x` is never transferred to the device.

Per core: DMA a host-replicated [128, 1024] f32 PE block (512 KiB) into
SBUF, then issue one DMA whose source AP has a step-0 (broadcast) middle
dim — each partition re-reads its 4 KiB row 64 times while writing a
contiguous 256 KiB DRAM span. One dma_start covers the whole 32 MiB
shard at near HBM line rate.
"""

import numpy as np

N_CORES = 8
BATCH = 65536
SEQ = 64
PATCH = 16
ROW = SEQ * PATCH                # 1024 f32 per batch element
SHARD = BATCH // N_CORES         # 8192 batch rows per core
REPS = SHARD // 128              # 64 rows per SBUF partition

_CACHE = {}


DUP = 2                          # copies of the row per SBUF partition
SEG = DUP * ROW                  # 2048 f32 = 8 KiB contiguous per descriptor
SEG_REPS = SHARD // (128 * DUP)  # 32 segment repeats per partition


def _pe2_block() -> np.ndarray:
    """[128, DUP*1024] f32: each partition holds DUP copies of 2*pe flattened.

    Computed with jax.numpy in f32 to match the reference bit-for-bit.
    """
    import jax
    import jax.numpy as jnp

    with jax.default_device(jax.devices("cpu")[0]):
        i = jnp.arange(SEQ, dtype=jnp.float32)[:, None]
        j = jnp.arange(PATCH, dtype=jnp.float32)[None, :]
        div = i / jnp.power(jnp.float32(10000.0), j / PATCH * 2.0)
        even = (jnp.arange(PATCH) % 2 == 0)[None, :]
        pe = jnp.where(even, jnp.sin(div), jnp.cos(div))
        row = np.asarray((2.0 * pe).astype(jnp.float32)).reshape(1, ROW)
    return np.broadcast_to(np.tile(row, (1, DUP)), (128, SEG)).copy()


def _build():
    from concourse import bacc, mybir

    nc = bacc.Bacc(
        "TRN2",
        target_bir_lowering=False,
        debug=False,
        enable_partition_id=False,
        monotonic_sem_count=0,
    )
    pe_in = nc.dram_tensor(
        "pe", [128, SEG], mybir.dt.float32, kind="ExternalInput"
    ).ap()
    out = nc.dram_tensor(
        "out", [SHARD, ROW], mybir.dt.float32, kind="ExternalOutput"
    ).ap()

    with (
        nc.sbuf_tensor([128, SEG], mybir.dt.float32) as t,
        nc.sbuf_tensor([128, 16], mybir.dt.float32) as warm,
        nc.semaphore() as dma_sem,
        nc.semaphore() as warm_sem,
        nc.Block() as block,
    ):
        # Output as [128 partitions, SEG_REPS segments, SEG floats]; split
        # the segment dim across the two HWDGE rings (SP + ACT) so both
        # sequencers feed the SDMA engines concurrently.
        half = SEG_REPS // 2
        src_half = t[:].unsqueeze(1).broadcast_to([128, half, SEG])
        dst = out.rearrange("(p r q) f -> p r (q f)", p=128, q=DUP)

        @block.sync
        def _(sync):
            sync.dma_start(t[:], pe_in).then_inc(dma_sem, 16)
            sync.wait_ge(dma_sem, 16)
            sync.dma_start(dst[:, :half], src_half).then_inc(dma_sem, 16)
            sync.wait_ge(dma_sem, 48)

        @block.scalar
        def _(scalar):
            # Cold-start qActDynamicHW during the input phase so output
            # descriptors flow immediately once the wait clears.
            scalar.dma_start(warm[:1, :], pe_in[:1, :16]).then_inc(warm_sem, 16)
            scalar.wait_ge(dma_sem, 16)
            scalar.dma_start(dst[:, half:], src_half).then_inc(dma_sem, 16)
            scalar.wait_ge(dma_sem, 48)

    nc.compile()
    return nc


def _get_nc():
    if "nc" not in _CACHE:
        _CACHE["nc"] = _build()
    return _CACHE["nc"]


def run_on_device(trace: bool = False, **kwargs):
    """Compile + run the SPMD kernel on all 8 cores; returns BassKernelResults."""
    from concourse.bass_utils import run_bass_kernel_spmd

    nc = _get_nc()
    block = _pe2_block()
    in_maps = [{"pe": block} for _ in range(N_CORES)]
    return run_bass_kernel_spmd(
        nc, in_maps, core_ids=list(range(N_CORES)), trace=trace, **kwargs
    )


def kernel(**inputs: np.ndarray) -> np.ndarray:
    x = inputs["x"]
    assert x.shape == (BATCH, SEQ, PATCH), x.shape
    res = run_on_device()
    shards = [r["out"].reshape(SHARD, SEQ, PATCH) for r in res.results]
    return np.concatenate(shards, axis=0).astype(np.float32)


# revision 13
# speedup vs baseline: 1.1177x; 1.1177x over previous
"""Trainium2 Bass kernel for nn_PositionalEncoding.

The reference module's realized math discards the input entirely:
    out = broadcast_to(2 * pe_table, x.shape)   # pe_table is [64, 16]
so the kernel is pure output-bandwidth bound. Each of the 8 cores writes
its (8192, 64, 16) f32 shard (32 MiB) of the broadcast table. The input
`x` is never transferred to the device.

Per core: DMA a host-replicated [128, 1024] f32 PE block (512 KiB) into
SBUF, then issue one DMA whose source AP has a step-0 (broadcast) middle
dim — each partition re-reads its 4 KiB row 64 times while writing a
contiguous 256 KiB DRAM span. One dma_start covers the whole 32 MiB
shard at near HBM line rate.
"""

import numpy as np

N_CORES = 8
BATCH = 65536
SEQ = 64
PATCH = 16
ROW = SEQ * PATCH                # 1024 f32 per batch element
SHARD = BATCH // N_CORES         # 8192 batch rows per core
REPS = SHARD // 128              # 64 rows per SBUF partition

_CACHE = {}


DUP = 2                          # copies of the row per SBUF partition
SEG = DUP * ROW                  # 2048 f32 = 8 KiB contiguous per descriptor
SEG_REPS = SHARD // (128 * DUP)  # 32 segment repeats per partition


def _pe2_block() -> np.ndarray:
    """[128, DUP*1024] f32: each partition holds DUP copies of 2*pe flattened.

    Computed with jax.numpy in f32 to match the reference bit-for-bit.
    """
    import jax
    import jax.numpy as jnp

    with jax.default_device(jax.devices("cpu")[0]):
        i = jnp.arange(SEQ, dtype=jnp.float32)[:, None]
        j = jnp.arange(PATCH, dtype=jnp.float32)[None, :]
        div = i / jnp.power(jnp.float32(10000.0), j / PATCH * 2.0)
        even = (jnp.arange(PATCH) % 2 == 0)[None, :]
        pe = jnp.where(even, jnp.sin(div), jnp.cos(div))
        row = np.asarray((2.0 * pe).astype(jnp.float32)).reshape(1, ROW)
    return np.broadcast_to(np.tile(row, (1, DUP)), (128, SEG)).copy()


def _build():
    from concourse import bacc, mybir

    nc = bacc.Bacc(
        "TRN2",
        target_bir_lowering=False,
        debug=False,
        enable_partition_id=False,
        monotonic_sem_count=0,
    )
    pe_in = nc.dram_tensor(
        "pe", [128, SEG], mybir.dt.float32, kind="ExternalInput"
    ).ap()
    out = nc.dram_tensor(
        "out", [SHARD, ROW], mybir.dt.float32, kind="ExternalOutput"
    ).ap()

    # Segments of the output covered by the direct HBM->HBM head-start DMA
    # (it runs during the otherwise-dead input-load window).
    K_HBM = 4

    with (
        nc.sbuf_tensor([128, SEG], mybir.dt.float32) as t,
        nc.semaphore() as dma_sem,
        nc.Block() as block,
    ):
        # Output viewed as [128 partition-blocks, SEG_REPS segments, SEG f32].
        dst = out.rearrange("(p r q) f -> p r (q f)", p=128, q=DUP)
        kb = SEG_REPS - K_HBM
        src_sbuf = t[:].unsqueeze(1).broadcast_to([128, kb, SEG])
        src_hbm = pe_in.unsqueeze(1).broadcast_to([128, K_HBM, SEG])

        @block.sync
        def _(sync):
            sync.dma_start(t[:], pe_in).then_inc(dma_sem, 16)
            sync.wait_ge(dma_sem, 16)
            sync.dma_start(dst[:, :kb], src_sbuf).then_inc(dma_sem, 16)
            sync.wait_ge(dma_sem, 48)

        @block.scalar
        def _(scalar):
            # No data dependency: source is the pe table in DRAM itself.
            # Streams during the input-load window; also warms this ring.
            scalar.dma_start(dst[:, kb:], src_hbm).then_inc(dma_sem, 16)
            scalar.wait_ge(dma_sem, 48)

    nc.compile()
    return nc


def _get_nc():
    if "nc" not in _CACHE:
        _CACHE["nc"] = _build()
    return _CACHE["nc"]


def run_on_device(trace: bool = False, **kwargs):
    """Compile + run the SPMD kernel on all 8 cores; returns BassKernelResults."""
    from concourse.bass_utils import run_bass_kernel_spmd

    nc = _get_nc()
    block = _pe2_block()
    in_maps = [{"pe": block} for _ in range(N_CORES)]
    return run_bass_kernel_spmd(
        nc, in_maps, core_ids=list(range(N_CORES)), trace=trace, **kwargs
    )


def kernel(**inputs: np.ndarray) -> np.ndarray:
    x = inputs["x"]
    assert x.shape == (BATCH, SEQ, PATCH), x.shape
    res = run_on_device()
    shards = [r["out"].reshape(SHARD, SEQ, PATCH) for r in res.results]
    return np.concatenate(shards, axis=0).astype(np.float32)
